# revision 12
# baseline (speedup 1.0000x reference)
"""GGNN (Devign) message-passing kernel for 8 Trainium2 NeuronCores.

Strategy (graph-parallel, dst-sharded, scatter-free):
  - Nodes are sharded across 8 cores at graph boundaries (32 graphs/core).
    Each core renumbers its SH columns into per-core in-degree-sorted
    "position space" (pos 0 reserved as a guaranteed-zero pad column, one
    junk column pinned at pos SH-1 for the second-half pad row).
  - Per step: each core computes m = h @ W for its shard (position-major,
    padded 256-col bf16 rows), AllGathers the two half-tables, gathers the
    rows for its in-edges with bulk dma_gather (two int16-indexed views,
    split by source row < / >= HS), segment-sums them with log-depth DVE
    folds over degree-sorted ELL layouts, adds the two structure partials,
    and writes the result CONTIGUOUSLY to a DRAM table (positions are
    already in output order -> no dma_scatter_add). The table is
    DMA-transposed into feature-major SBUF and the GRU runs with bf16
    matmuls (biases folded in via a masked ones-row at K index 96 of
    block 1, which also keeps junk columns exactly zero).
  - After 6 steps: transpose h back to node-major via an identity matmul,
    regroup columns graph-contiguously with one transpose-mode dma_gather,
    per-graph reduce_max pooling, ReLU, classifier, sigmoid -> [2, 32].
"""

import math

import ml_dtypes
import numpy as np

P = 128
NC = 8
D = 200
DP = 256          # padded feature dim (bf16 rows = 512B, dma_gather elem)
NSTEP = 6
NG = 256
GPC = NG // NC    # graphs per core
N_NODES = 50000
N_EDGES = 400000
ONES_ROW = 96     # partition index (block 1) of the ones (bias/mask) K row
SLOT_BUDGET = 18  # max blocks (of [128, 256] bf16) per gather tile

BF16 = ml_dtypes.bfloat16


def _wrap_idx(arr):
    """Linear int16 index list -> [128, len/16] SBUF layout (16-part wrap,
    replicated 8x for the Q7 cores)."""
    assert arr.size % 16 == 0
    w = arr.reshape(-1, 16).T.astype(np.int16)  # [16, L/16]
    return np.tile(w, (8, 1))                   # [128, L/16]


def _preprocess(x, edge_index, batch):
    """Build per-core inputs + compile-time structure shared by all cores."""
    batch = np.asarray(batch).astype(np.int64)
    src = np.asarray(edge_index[0]).astype(np.int64)
    dst = np.asarray(edge_index[1]).astype(np.int64)
    n = batch.shape[0]

    node_core = batch // GPC
    gcounts = np.bincount(batch, minlength=NG).reshape(NC, GPC)
    gsize = gcounts.max(axis=0)                      # [GPC] cross-core aligned
    goff = 1 + np.concatenate([[0], np.cumsum(gsize)[:-1]])
    used = int(1 + gsize.sum())
    SH = ((used + 511) // 512) * 512
    HS = SH // 2
    assert 8 * HS <= 32768

    # node -> local slot (graph-aligned)
    first_of_graph = np.searchsorted(batch, np.arange(NG))
    rank = np.arange(n) - first_of_graph[batch]
    slot = goff[batch % GPC] + rank                  # [n], in [1, used)

    dst_core = node_core[dst]
    src_core = node_core[src]
    sslot = slot[src]
    dslot = slot[dst]

    # per-core in-degree per slot
    indeg = np.zeros((NC, SH), np.int64)
    for c in range(NC):
        indeg[c] = np.bincount(dslot[dst_core == c], minlength=SH)

    # real-slot mask per core (slot occupied by a real node on that core)
    real = np.zeros((NC, SH), bool)
    for c in range(NC):
        nodes_c = np.nonzero(node_core == c)[0]
        real[c, slot[nodes_c]] = True

    # per-core position permutation: pos 0 = slot 0 (always junk, zero);
    # pos SH-1 = a junk slot (guaranteed-zero pad row for structure 1);
    # remaining slots sorted by in-degree descending.
    pos_of_slot = np.zeros((NC, SH), np.int64)
    slot_of_pos = np.zeros((NC, SH), np.int64)
    for c in range(NC):
        junk = np.nonzero(~real[c])[0]
        junk1 = int(junk[junk != 0][0])
        order = np.argsort(-indeg[c], kind="stable")
        order = order[(order != 0) & (order != junk1)]
        sop = np.concatenate([[0], order, [junk1]])
        slot_of_pos[c] = sop
        pos_of_slot[c, sop] = np.arange(SH)

    # Per-edge: table row (in source core's position space) + structure.
    pos_src = pos_of_slot[src_core, sslot]
    s_e = (pos_src >= HS).astype(np.int64)
    row_e = src_core * HS + pos_src - s_e * HS       # int16-safe (< 8*HS)
    pad_row = (0, HS - 1)                            # guaranteed-zero rows

    # Per (core, structure): counts per dst position, ELL caps.
    deg2 = np.zeros((NC, 2, SH), np.int64)
    edges_by_core = {}
    for c in range(NC):
        m = dst_core == c
        p_e = pos_of_slot[c, dslot[m]]
        se = s_e[m]
        re_ = row_e[m]
        edges_by_core[c] = (p_e, se, re_)
        for s in range(2):
            deg2[c, s] = np.bincount(p_e[se == s], minlength=SH)

    NCHU = (used + P - 1) // P                       # live chunks (pos < used)
    Kch = deg2.reshape(NC, 2, SH // P, P).max(axis=3).max(axis=0)  # [2, nch]
    nch_live = 0
    for ch in range(SH // P):
        if Kch[0, ch] or Kch[1, ch]:
            nch_live = ch + 1
    assert nch_live <= NCHU + 1

    # Group packing: consecutive chunks; per-structure tile budget.
    groups = []                                      # (ch0, g, k0, k1)
    ch = 0
    while ch < nch_live:
        k0 = int(Kch[0, ch])
        k1 = int(Kch[1, ch])
        g = 1
        while ch + g < nch_live:
            nk0 = max(k0, int(Kch[0, ch + g]))
            nk1 = max(k1, int(Kch[1, ch + g]))
            if (g + 1) * max(nk0, 1) > SLOT_BUDGET or (g + 1) * max(nk1, 1) > SLOT_BUDGET:
                break
            k0, k1 = nk0, nk1
            g += 1
        groups.append((ch, g, k0, k1))
        ch += g

    # Build per-core ELL matrices [SH, k] of view-local rows per structure.
    kmax = [max((int(Kch[s, ch]) for ch in range(nch_live)), default=1) for s in range(2)]
    ells = np.zeros((NC, 2, SH, max(max(kmax), 1)), np.int64)
    for s in range(2):
        ells[:, s, :, :] = pad_row[s]
    for c in range(NC):
        p_e, se, re_ = edges_by_core[c]
        for s in range(2):
            ms = se == s
            pe = p_e[ms]
            rr = re_[ms]
            order = np.argsort(pe, kind="stable")
            pe, rr = pe[order], rr[order]
            starts = np.searchsorted(pe, np.arange(SH))
            kk = np.arange(pe.size) - starts[pe]
            ells[c, s, pe, kk] = rr

    gidx_parts = {c: [] for c in range(NC)}
    goffsets = []       # per group: (col offset s0, col offset s1)
    gcol = 0
    for (ch0, g, k0, k1) in groups:
        offs = []
        for s, k in ((0, k0), (1, k1)):
            offs.append(gcol)
            if k > 0:
                for c in range(NC):
                    sl = ells[c, s, ch0 * P:(ch0 + g) * P, :k]    # [g*128, k]
                    gidx_parts[c].append(
                        _wrap_idx(sl.T.reshape(-1).astype(np.int16)))
                gcol += (g * k * P) // 16
        goffsets.append(tuple(offs))

    per_core = []
    for c in range(NC):
        gi = (np.concatenate(gidx_parts[c], axis=1)
              if gidx_parts[c] else np.zeros((128, 16), np.int16))
        # final pooling permutation: slot order -> position
        gf = _wrap_idx(pos_of_slot[c].astype(np.int16))
        # initial hT (feature-major, bf16, position space) and column mask
        hT0 = np.zeros((P, 2, SH), np.float32)
        mask = np.zeros((1, 2, SH), np.float32)
        nodes_c = np.nonzero(node_core == np.int64(c))[0]
        pc_ = pos_of_slot[c, slot[nodes_c]]
        xc = np.asarray(x)[nodes_c]                  # [n_c, 200]
        hT0[:, 0, pc_] = xc[:, 0:128].T
        hT0[0:D - 128, 1, pc_] = xc[:, 128:D].T
        mask[0, :, pc_] = 1.0
        hT0[ONES_ROW, 1, :] = mask[0, 0, :]
        per_core.append(dict(
            hT0=hT0.astype(BF16),
            gidx=gi.astype(np.int16),
            gidxF=gf.astype(np.int16),
        ))

    meta = dict(SH=SH, HS=HS, NCH=SH // P, groups=groups, goffsets=goffsets,
                gsize=gsize, goff=goff, nch_live=nch_live,
                gidx_cols=max(gcol, 16), gidxF_cols=SH // 16)
    return per_core, meta


def _prep_weights(ggnn_weight, w_ih, w_hh, b_ih, b_hh, cls_w, cls_b):
    # wt[t] for t<NSTEP: GGNN step weight; wt[NSTEP]: identity (final
    # node-major transpose of h via the same matmul path).
    wt = np.zeros((NSTEP + 1, 2, P, DP), np.float32)
    for t in range(NSTEP):
        wt[t, 0, :, 0:D] = ggnn_weight[t][0:128, :]
        wt[t, 1, 0:D - 128, 0:D] = ggnn_weight[t][128:D, :]
    wt[NSTEP, 0, :, 0:128] = np.eye(128)
    wt[NSTEP, 1, 0:D - 128, 128:D] = np.eye(D - 128)

    def packT(w, b):
        o = np.zeros((2, P, 3 * D), np.float32)
        o[0, :, :] = w[:, 0:128].T
        o[1, 0:D - 128, :] = w[:, 128:D].T
        o[1, ONES_ROW, :] = b
        return o.astype(BF16)

    wih = packT(np.asarray(w_ih), np.asarray(b_ih))
    whh = packT(np.asarray(w_hh), np.asarray(b_hh))
    wcls = np.zeros((2, P, 2), np.float32)
    wcls[0] = np.asarray(cls_w)[:, 0:128].T
    wcls[1, 0:D - 128] = np.asarray(cls_w)[:, 128:D].T
    return dict(
        wt=wt.astype(BF16), wih=wih, whh=whh,
        wcls=wcls.astype(BF16),
        bcls=np.asarray(cls_b).reshape(2, 1).astype(np.float32),
    )


def _build_program(meta, parts=frozenset(('lib', 'cc', 'gs', 'sh'))):
    import concourse.bacc as bacc
    import concourse.bass as bass
    import concourse.mybir as mybir
    import concourse.tile as tile
    from concourse.library_config import mlp

    SH, NCH = meta["SH"], meta["NCH"]
    HS = meta["HS"]
    groups, goffsets = meta["groups"], meta["goffsets"]
    gsize, goff = meta["gsize"], meta["goff"]
    nch_live = meta["nch_live"]
    NNC = SH // 512
    bf16, f32, i16 = mybir.dt.bfloat16, mybir.dt.float32, mybir.dt.int16
    AF = mybir.ActivationFunctionType

    nc = bacc.Bacc("TRN2")
    # --- I/O ---
    hT0_in = nc.dram_tensor("hT0", [P, 2, SH], bf16, kind="ExternalInput")
    gidx_in = nc.dram_tensor("gidx", [P, meta["gidx_cols"]], i16, kind="ExternalInput")
    gidxF_in = nc.dram_tensor("gidxF", [P, meta["gidxF_cols"]], i16, kind="ExternalInput")
    wt_in = nc.dram_tensor("wt", [NSTEP + 1, 2, P, DP], bf16, kind="ExternalInput")
    wih_in = nc.dram_tensor("wih", [2, P, 3 * D], bf16, kind="ExternalInput")
    whh_in = nc.dram_tensor("whh", [2, P, 3 * D], bf16, kind="ExternalInput")
    wcls_in = nc.dram_tensor("wcls", [2, P, 2], bf16, kind="ExternalInput")
    bcls_in = nc.dram_tensor("bcls", [2, 1], f32, kind="ExternalInput")
    out_d = nc.dram_tensor("out", [2, GPC], f32, kind="ExternalOutput")

    from contextlib import ExitStack
    with tile.TileContext(nc) as tc, ExitStack() as ctx:
        const = ctx.enter_context(tc.tile_pool(name="const", bufs=1))
        dram = ctx.enter_context(tc.tile_pool(name="dram", bufs=3, space="DRAM"))
        hpool = ctx.enter_context(tc.tile_pool(name="hpool", bufs=1))
        slotp = ctx.enter_context(tc.tile_pool(name="slotp", bufs=2))
        mpool = ctx.enter_context(tc.tile_pool(name="mpool", bufs=1))
        gpool = ctx.enter_context(tc.tile_pool(name="gpool", bufs=3))
        psum_rz = ctx.enter_context(tc.tile_pool(name="psum_rz", bufs=2, space="PSUM"))
        psum_hn = ctx.enter_context(tc.tile_pool(name="psum_hn", bufs=2, space="PSUM"))

        if 'lib' in parts:
            nc.gpsimd.load_library(mlp)

        # --- load constants ---
        wt_t = const.tile([P, (NSTEP + 1) * 2, DP], bf16)
        nc.sync.dma_start(wt_t[:], wt_in.rearrange("t b p e -> p (t b) e"))
        wih_t = const.tile([P, 2, 3 * D], bf16)
        nc.sync.dma_start(wih_t[:], wih_in.rearrange("b p m -> p b m"))
        whh_t = const.tile([P, 2, 3 * D], bf16)
        nc.sync.dma_start(whh_t[:], whh_in.rearrange("b p m -> p b m"))
        wcls_t = const.tile([P, 2, 2], bf16)
        nc.sync.dma_start(wcls_t[:], wcls_in.rearrange("b p m -> p b m"))
        bcls_t = const.tile([2, 1], f32)
        nc.sync.dma_start(bcls_t[:], bcls_in[:])
        gidx_t = const.tile([P, meta["gidx_cols"]], i16)
        nc.sync.dma_start(gidx_t[:], gidx_in[:])
        gidxF_t = const.tile([P, meta["gidxF_cols"]], i16)
        nc.sync.dma_start(gidxF_t[:], gidxF_in[:])

        TAILB = SH // P - nch_live       # tail chunks never written by groups
        if TAILB > 0:
            ztail = const.tile([P, TAILB, DP], bf16)
            nc.vector.memset(ztail[:], 0.0)

        hA = hpool.tile([P, 2, SH], bf16, name="hA")
        hB = hpool.tile([P, 2, SH], bf16, name="hB")
        nc.sync.dma_start(hA[:], hT0_in[:])
        nc.vector.memset(hB[64:P, 1, :], 0.0)
        nc.vector.tensor_copy(hB[ONES_ROW:ONES_ROW + 1, 1, :],
                              hA[ONES_ROW:ONES_ROW + 1, 1, :])

        K1 = D - 128  # 72

        import os as _os
        REP = int(_os.environ.get("KREPEAT", "1"))
        for t in range(NSTEP * REP):
            h_old = hA if t % 2 == 0 else hB
            h_new = hB if t % 2 == 0 else hA

            # Interleaved table: local row 2j   = position j        (< HS)
            #                    local row 2j+1 = position HS + j
            # so structure-s gather indices c*HS + j address the stride-2
            # view tab[s::2] and stay within int16 range.
            mbbA = dram.tile([SH, DP], bf16, name="mbbA", tag="mbbA")
            tab = dram.tile([NC * SH, DP], bf16, name="tab0", tag="tab0",
                            addr_space="Shared" if 'sh' in parts else "Local")
            tabs = [tab[0:2 * NC * HS:2, :], tab[1:2 * NC * HS:2, :]]
            sd = dram.tile([SH, DP], bf16, name="sd", tag="sd")
            if TAILB > 0:
                nc.sync.dma_start(
                    sd[nch_live * P:SH, :].rearrange("(c p) e -> p c e", p=P),
                    ztail[:])

            # --- m = h @ W_t  (position-major, bf16, padded cols) ---
            QCH = NCH // 4
            for q in range(4):
                par = 0 if q < 2 else 1      # structure parity of this quarter
                qh = q % 2
                mt = mpool.tile([P, QCH, DP], bf16, name="mtq", tag="mtq", bufs=2)
                for ch in range(QCH):
                    mp = psum_hn.tile([P, DP], f32, name="mp", tag="hn")
                    cols = slice((q * QCH + ch) * P, (q * QCH + ch + 1) * P)
                    nc.tensor.matmul(out=mp[:], lhsT=h_old[:, 0, cols],
                                     rhs=wt_t[:, (t % NSTEP) * 2, :], start=True, stop=False)
                    nc.tensor.matmul(out=mp[:], lhsT=h_old[0:K1, 1, cols],
                                     rhs=wt_t[0:K1, (t % NSTEP) * 2 + 1, :], start=False, stop=True)
                    nc.any.tensor_copy(mt[:, ch, :], mp[:])
                nc.sync.dma_start(
                    mbbA[2 * qh * QCH * P + par:2 * (qh + 1) * QCH * P:2, :]
                    .rearrange("(c p) e -> p c e", p=P), mt[:])

            # --- AllGather m (single merged collective) ---
            if 'cc' in parts:
                nc.gpsimd.collective_compute(
                    "AllGather", mybir.AluOpType.bypass,
                    replica_groups=[list(range(NC))],
                    ins=[mbbA.opt()], outs=[tab.opt()],
                )

            # --- gather + fold per group; contiguous write (no scatter) ---
            # Phase A: structure-0 gathers+folds (need only tab0) land in the
            # f0all staging tile, hiding under AG_B's flight. Phase B merges
            # structure-1 and writes sd.
            _ng = int(_os.environ.get("KGROUPS", "999"))
            f0all = mpool.tile([P, nch_live, DP], bf16, name="f0all", tag="f0all")
            live_groups = (groups if 'gs' in parts else [])[:_ng]
            for gi_, (ch0, g, k0, k1) in enumerate(live_groups):
                col0, col1 = goffsets[gi_]
                if k0 == 0:
                    nc.vector.memset(f0all[:, ch0:ch0 + g, :], 0.0)
                    continue
                nidx = g * k0 * P
                tg = slotp.tile([P, SLOT_BUDGET, DP], bf16, name="tg0", tag="tg0")
                nc.gpsimd.dma_gather(
                    tg[:, 0:g * k0, :], tabs[0],
                    gidx_t[:, col0:col0 + nidx // 16],
                    nidx, nidx, DP, elem_step=2 * DP, single_packet=False,
                )
                kk = k0
                while kk > 2:
                    hi = (kk + 1) // 2
                    lo = kk // 2
                    nc.vector.tensor_add(
                        out=tg[:, 0:lo * g, :],
                        in0=tg[:, 0:lo * g, :],
                        in1=tg[:, hi * g:(hi + lo) * g, :],
                    )
                    kk = hi
                if kk == 2:
                    nc.vector.tensor_add(out=f0all[:, ch0:ch0 + g, :],
                                         in0=tg[:, 0:g, :], in1=tg[:, g:2 * g, :])
                else:
                    nc.vector.tensor_copy(f0all[:, ch0:ch0 + g, :], tg[:, 0:g, :])
            for gi_, (ch0, g, k0, k1) in enumerate(live_groups):
                col0, col1 = goffsets[gi_]
                if k1 > 0:
                    nidx = g * k1 * P
                    tg = slotp.tile([P, SLOT_BUDGET, DP], bf16, name="tg1", tag="tg1")
                    nc.gpsimd.dma_gather(
                        tg[:, 0:g * k1, :], tabs[1],
                        gidx_t[:, col1:col1 + nidx // 16],
                        nidx, nidx, DP, elem_step=2 * DP, single_packet=False,
                    )
                    kk = k1
                    while kk > 1:
                        hi = (kk + 1) // 2
                        lo = kk // 2
                        nc.vector.tensor_add(
                            out=tg[:, 0:lo * g, :],
                            in0=tg[:, 0:lo * g, :],
                            in1=tg[:, hi * g:(hi + lo) * g, :],
                        )
                        kk = hi
                    nc.vector.tensor_add(out=tg[:, 0:g, :],
                                         in0=tg[:, 0:g, :],
                                         in1=f0all[:, ch0:ch0 + g, :])
                    res, roff = tg, 0
                else:
                    res, roff = f0all, ch0
                nc.sync.dma_start(
                    sd[ch0 * P:(ch0 + g) * P, :].rearrange("(c p) e -> p c e", p=P),
                    res[:, roff:roff + g, :])

            # --- transpose agg to feature-major ---
            aggT = mpool.tile([P, 2, SH], bf16, name="aggT", tag="aggT")
            nc.sync.dma_start(out=aggT[:, 0, :], in_=sd[0:SH, 0:128],
                              transpose=True)
            nc.sync.dma_start(out=aggT[:, 1, :], in_=sd[0:SH, 128:256],
                              transpose=True)
            nc.vector.tensor_copy(aggT[ONES_ROW:ONES_ROW + 1, 1, :],
                                  h_old[ONES_ROW:ONES_ROW + 1, 1, :])

            # --- GRU ---
            for i in range(NNC):
                cols = slice(i * 512, (i + 1) * 512)
                for mb in range(2):
                    mr = P if mb == 0 else K1
                    gsl = [slice(gg * D + mb * 128, gg * D + mb * 128 + mr) for gg in range(3)]
                    rz = psum_rz.tile([P, 1024], f32, name="rz", tag="rz")
                    for half, gg in ((0, 0), (1, 1)):  # r, z gates
                        o = rz[0:mr, half * 512:(half + 1) * 512]
                        nc.tensor.matmul(out=o, lhsT=wih_t[:, 0, gsl[gg]],
                                         rhs=aggT[:, 0, cols], start=True, stop=False)
                        nc.tensor.matmul(out=o, lhsT=wih_t[0:ONES_ROW + 1, 1, gsl[gg]],
                                         rhs=aggT[0:ONES_ROW + 1, 1, cols], start=False, stop=False)
                        nc.tensor.matmul(out=o, lhsT=whh_t[:, 0, gsl[gg]],
                                         rhs=h_old[:, 0, cols], start=False, stop=False)
                        nc.tensor.matmul(out=o, lhsT=whh_t[0:ONES_ROW + 1, 1, gsl[gg]],
                                         rhs=h_old[0:ONES_ROW + 1, 1, cols], start=False, stop=True)
                    rzs = gpool.tile([P, 1024], bf16, name="rzs", tag="rzs")
                    nc.scalar.activation(rzs[0:mr, :], rz[0:mr, :], AF.Sigmoid)

                    hn = psum_hn.tile([P, 1024], f32, name="hn", tag="hn")
                    nc.tensor.matmul(out=hn[0:mr, 0:512], lhsT=whh_t[:, 0, gsl[2]],
                                     rhs=h_old[:, 0, cols], start=True, stop=False)
                    nc.tensor.matmul(out=hn[0:mr, 0:512], lhsT=whh_t[0:ONES_ROW + 1, 1, gsl[2]],
                                     rhs=h_old[0:ONES_ROW + 1, 1, cols], start=False, stop=True)
                    nc.tensor.matmul(out=hn[0:mr, 512:1024], lhsT=wih_t[:, 0, gsl[2]],
                                     rhs=aggT[:, 0, cols], start=True, stop=False)
                    nc.tensor.matmul(out=hn[0:mr, 512:1024], lhsT=wih_t[0:ONES_ROW + 1, 1, gsl[2]],
                                     rhs=aggT[0:ONES_ROW + 1, 1, cols], start=False, stop=True)

                    hns = gpool.tile([P, 512], bf16, name="hns", tag="hns")
                    nc.any.tensor_copy(hns[0:mr, :], hn[0:mr, 0:512])
                    rhn = gpool.tile([P, 512], bf16, name="rhn", tag="rhn")
                    nc.vector.tensor_mul(rhn[0:mr, :], rzs[0:mr, 0:512], hns[0:mr, :])
                    nc.vector.tensor_add(hn[0:mr, 512:1024], hn[0:mr, 512:1024],
                                         rhn[0:mr, :])
                    nt = gpool.tile([P, 512], bf16, name="nt", tag="nt")
                    nc.scalar.activation(nt[0:mr, :], hn[0:mr, 512:1024], AF.Tanh)

                    t1 = gpool.tile([P, 512], bf16, name="t1", tag="t1")
                    nc.vector.tensor_sub(t1[0:mr, :], h_old[0:mr, mb, cols], nt[0:mr, :])
                    nc.vector.tensor_mul(t1[0:mr, :], rzs[0:mr, 512:1024], t1[0:mr, :])
                    nc.vector.tensor_add(h_new[0:mr, mb, cols], nt[0:mr, :], t1[0:mr, :])

        # --- final: node-major h via identity matmul, pooling permutation ---
        h_fin = hA if (NSTEP * REP) % 2 == 0 else hB
        hd = dram.tile([SH, DP], bf16, name="hd", tag="sd")
        QCH = NCH // 4
        for q in range(4):
            mt = mpool.tile([P, QCH, DP], bf16, name="mtq", tag="mtq", bufs=2)
            for ch in range(QCH):
                mp = psum_hn.tile([P, DP], f32, name="mp", tag="hn")
                cols = slice((q * QCH + ch) * P, (q * QCH + ch + 1) * P)
                nc.tensor.matmul(out=mp[:], lhsT=h_fin[:, 0, cols],
                                 rhs=wt_t[:, NSTEP * 2, :], start=True, stop=False)
                nc.tensor.matmul(out=mp[:], lhsT=h_fin[0:K1, 1, cols],
                                 rhs=wt_t[0:K1, NSTEP * 2 + 1, :], start=False, stop=True)
                nc.any.tensor_copy(mt[:, ch, :], mp[:])
            nc.sync.dma_start(
                hd[q * QCH * P:(q + 1) * QCH * P, :].rearrange("(c p) e -> p c e", p=P),
                mt[:])

        hTg = mpool.tile([P, 2, SH], bf16, name="aggT", tag="aggT")
        nc.gpsimd.dma_gather(
            hTg[:], hd[0:SH, :], gidxF_t[:],
            SH, SH, DP, transpose=True, single_packet=False,
        )

        pooled = gpool.tile([P, 2, GPC], bf16, name="pooled", bufs=1)
        for g in range(GPC):
            a, b = int(goff[g]), int(goff[g] + gsize[g])
            nc.vector.tensor_reduce(pooled[:, :, g:g + 1], hTg[:, :, a:b],
                                    axis=mybir.AxisListType.X,
                                    op=mybir.AluOpType.max)
        nc.vector.tensor_scalar_max(pooled[:], pooled[:], 0.0)
        lg = psum_hn.tile([2, GPC], f32, name="lg", tag="hn")
        nc.tensor.matmul(out=lg[:], lhsT=wcls_t[:, 0, :], rhs=pooled[:, 0, :],
                         start=True, stop=False)
        nc.tensor.matmul(out=lg[:], lhsT=wcls_t[0:K1, 1, :], rhs=pooled[0:K1, 1, :],
                         start=False, stop=True)
        ot = gpool.tile([2, GPC], f32, name="ot", bufs=1)
        nc.scalar.activation(ot[:], lg[:], AF.Sigmoid, bias=bcls_t[:])
        nc.sync.dma_start(out_d[:], ot[:])

    nc.compile()
    return nc


def kernel(**inputs):
    import os
    from concourse.bass_utils import run_bass_kernel_spmd

    per_core, meta = _preprocess(inputs["x"], inputs["edge_index"], inputs["batch"])
    w = _prep_weights(inputs["ggnn_weight"], inputs["w_ih"], inputs["w_hh"],
                      inputs["b_ih"], inputs["b_hh"], inputs["cls_w"], inputs["cls_b"])
    in_maps = [dict(hT0=pc["hT0"], gidx=pc["gidx"], gidxF=pc["gidxF"], **w)
               for pc in per_core]
    nc = _build_program(meta)
    trace = bool(int(os.environ.get("KTRACE", "0")))
    res = run_bass_kernel_spmd(nc, in_maps, core_ids=list(range(NC)), trace=trace)
    if trace:
        print(f"HW exec time: {res.exec_time_ns} ns")
        print("trace:", res.instructions_and_trace[1] if res.instructions_and_trace else None)
    out = np.zeros((NG, 2), np.float32)
    for c in range(NC):
        out[c * GPC:(c + 1) * GPC, :] = res.results[c]["out"].T
    return out
